# revision 40
# baseline (speedup 1.0000x reference)
"""Trainium2 Bass kernel for nn_DLUPack (CARAFE-style dynamic upsampling).

Sharding: 8 cores = (batch n in [0,4)) x (output-row-parity s in {0,1});
core (n, s) computes low-res rows hh in [32s, 32s+32) -> all parity-s output rows.

Reference output mapping (its reshape scrambles positions):
  ref[n, c, 2y+i, 2x+j] = sum_k patches[c, hh, ww, k] * kern[hh, ww, k, u]
  with hh = 32s + 16jh + m:  row r = 8m + 2(ww//16) + s, col = 8*(ww%16) + 2u + jh.

Device pipeline per core (128-partition mid-section: p = 64*jh + w):
  1. compressor 1x1 conv (PE, fp16) -> cx [64, 38, 66]
  2. offset+mask 3x3 convs (9 accumulated MMs) -> psum [57, .]: off ch 0-7, mask ch 32-56
  3. exp in ACT evac -> expS [25, 36, 64]; PE-transpose row-pairs (r, r+16)
     -> expT2 [128 = jh*64+w, 20 hl, 25 k]; softmax via free-dim reduce
  4. offset PE-transpose pairs (m, m+16) -> deltT2 [128, 16, 8]; W9 weights (DVE)
  5. kernc assembly [128, 16m, 25k, 4u]: 9 broadcast-multiply terms (DVE)
  6. kernc block-shifted +-1,+-2 via SBUF-SBUF DMA (800B runs); prep slices -> data_all
  7. per pair m: 2 local_scatter (GPSIMD) -> banded [128, 3*512 + 2*512]
  8. carafe: 5 accumulated MMs [128,128]x[128,512] per (pair, c-half) -> psum [128,512]
  9. ACT evac (fp16) -> DMA out 4 contiguous output rows (host converts to f32)
"""
import sys
import numpy as np

sys.path.insert(0, '/opt/trn_rl_repo')

import ml_dtypes  # noqa: E402
from contextlib import ExitStack  # noqa: E402

import concourse.bass as bass  # noqa: E402
import concourse.tile as tile  # noqa: E402
from concourse import mybir, bacc  # noqa: E402
from concourse.bass_utils import run_bass_kernel_spmd  # noqa: E402

F32 = mybir.dt.float32
BF16 = mybir.dt.float16  # NOTE: fp16 (better mantissa), name kept for brevity
I16 = mybir.dt.int16
AF = mybir.ActivationFunctionType
OP = mybir.AluOpType

N, C, H, W = 4, 256, 64, 64


def _ap(base, off_elems, dims):
    return bass.AP(tensor=base.tensor, offset=base.offset + off_elems, ap=[list(d) for d in dims])


def build_scatter_tables():
    idx1 = -np.ones((128, 100), np.int16)
    idx2 = -np.ones((128, 100), np.int16)
    for p in range(128):
        jh, wpp = p // 64, p % 64
        for b in range(5):
            w = wpp + b - 2
            if not (0 <= w < 64):
                continue
            q, wl = w // 16, w % 16
            for ki in range(5):
                for u in range(4):
                    col = q * 128 + 8 * wl + 2 * u + jh
                    qidx = (b * 5 + ki) * 4 + u
                    if ki < 3:
                        idx1[p, qidx] = ki * 512 + col
                    else:
                        idx2[p, qidx] = (ki - 3) * 512 + col
    return idx1, idx2


def build_program():
    nc = bacc.Bacc(None, target_bir_lowering=False, debug=True)

    xwin = nc.declare_dram_parameter('xwin', [2, 128, 38 * 64], BF16, isOutput=False)
    xT2 = nc.declare_dram_parameter('xT2', [128, 20 * 256], BF16, isOutput=False)
    wc = nc.declare_dram_parameter('wc', [128, 2 * 64], BF16, isOutput=False)
    wk = nc.declare_dram_parameter('wk', [128, 9 * 57], BF16, isOutput=False)
    bco = nc.declare_dram_parameter('bco', [57, 1], F32, isOutput=False)
    bcomp = nc.declare_dram_parameter('bcomp', [64, 1], F32, isOutput=False)
    wvec = nc.declare_dram_parameter('wvec', [128, 1], F32, isOutput=False)
    w63 = nc.declare_dram_parameter('w63', [128, 1], F32, isOutput=False)
    hrow = nc.declare_dram_parameter('hrow', [128, 16], F32, isOutput=False)
    y63 = nc.declare_dram_parameter('y63', [128, 16], F32, isOutput=False)
    ident = nc.declare_dram_parameter('ident', [128, 128], F32, isOutput=False)
    idx1 = nc.declare_dram_parameter('idx1', [128, 100], I16, isOutput=False)
    idx2 = nc.declare_dram_parameter('idx2', [128, 100], I16, isOutput=False)
    outp = nc.declare_dram_parameter('outp', [256, 64 * 128], BF16, isOutput=True)

    with tile.TileContext(nc) as tc, ExitStack() as ctx:
        sing = ctx.enter_context(tc.tile_pool(name='sing', bufs=1))
        work = ctx.enter_context(tc.tile_pool(name='work', bufs=1))
        band = ctx.enter_context(tc.tile_pool(name='band', bufs=4))
        rowp = ctx.enter_context(tc.tile_pool(name='rowp', bufs=4))
        psum = ctx.enter_context(tc.psum_pool(name='ps', bufs=2))
        psc = ctx.enter_context(tc.psum_pool(name='psc', bufs=4))

        def load(shape, dtype, src, eng):
            t = sing.tile(shape, dtype, name=f'ld_{src.tensor.name if hasattr(src, "tensor") else id(src)}')
            eng.dma_start(out=t[:], in_=src[:])
            return t

        # critical small loads first so warm-up + compressor can begin
        id_sb = load([128, 128], F32, ident, nc.sync)
        wc_sb = load([128, 2, 64], BF16, wc, nc.sync)
        xwin_sb = sing.tile([128, 2, 38 * 64], BF16)
        for grp in range(5):
            g0 = grp * 8
            rows = min(8, 38 - g0)
            for cg_ in range(2):
                nc.sync.dma_start(
                    out=_ap(xwin_sb[:], cg_ * 2432 + g0 * 64,
                            [[4864, 128], [1, rows * 64]]),
                    in_=_ap(xwin[:], cg_ * 128 * 2432 + g0 * 64,
                            [[2432, 128], [1, rows * 64]]))
        wk_sb = load([128, 9 * 57], BF16, wk, nc.scalar)
        xT2_sb = load([128, 20 * 256], BF16, xT2, nc.scalar)
        bco_sb = load([57, 1], F32, bco, nc.gpsimd)
        bcomp_sb = load([64, 1], F32, bcomp, nc.gpsimd)
        wvec_sb = load([128, 1], F32, wvec, nc.gpsimd)
        w63_sb = load([128, 1], F32, w63, nc.gpsimd)
        hrow_sb = load([128, 16], F32, hrow, nc.gpsimd)
        y63_sb = load([128, 16], F32, y63, nc.gpsimd)
        idx1_sb = load([128, 100], I16, idx1, nc.gpsimd)
        idx2_sb = load([128, 100], I16, idx2, nc.gpsimd)

        # msm4 shifted variants; cross-block rows are killed by zero edge weights,
        # only the truly unwritten end partitions need zeroing (NaN hygiene)
        msm4_p1 = work.tile([128, 500], BF16)   # msm4_p1[p] = msm4[p+1]
        msm4_m1 = work.tile([128, 500], BF16)   # msm4_m1[p] = msm4[p-1]
        nc.vector.memset(msm4_p1[:], 0.0)
        nc.vector.memset(msm4_m1[:], 0.0)

        # kernc block-shift buffers; memset once so block-edge partitions stay zero
        kbf = {}
        for d in (-2, -1, 1, 2):
            kbf[d] = work.tile([128, 1600], BF16, name=f'kbf{d}')
            nc.vector.memset(kbf[d][:], 0.0)

        # PE warm-up: keep TensorE busy during input-DMA wait (DVFS ramp)
        pw = psc.tile([128, 512], F32, name='pcs_warm', tag='pcs')
        for _ in range(32):
            nc.tensor.matmul(pw[0:64, 0:64], id_sb[:, 0:64], id_sb[:, 0:64], start=True, stop=True)

        # ---- 1. compressor ----
        # cx2: partitions 0-63 hold cx row r at slot r; partitions 64-127 hold row r+1
        # at slot r, so a 3x3-conv row pair (dy=0,1) contracts in one 128-deep matmul.
        cx_sb = work.tile([128, 38, 66], BF16)
        nc.vector.memset(_ap(cx_sb[:], 0, [[38 * 66, 128], [66, 38], [1, 1]]), 0.0)
        nc.vector.memset(_ap(cx_sb[:], 65, [[38 * 66, 128], [66, 38], [1, 1]]), 0.0)
        for grp in range(5):
            g0 = grp * 8
            rows = min(8, 38 - g0)
            nn = rows * 64
            pcs = psum.tile([64, 512], F32)
            for cg in range(2):
                nc.tensor.matmul(pcs[:, :nn], wc_sb[:, cg, :],
                                 xwin_sb[:, cg, g0 * 64:g0 * 64 + nn],
                                 start=(cg == 0), stop=(cg == 1))
            nc.scalar.activation(
                out=_ap(cx_sb[:], g0 * 66 + 1, [[38 * 66, 64], [66, rows], [1, 64]]),
                in_=_ap(pcs[:], 0, [[512, 64], [64, rows], [1, 64]]),
                func=AF.Identity, bias=bcomp_sb[:], scale=1.0)
            s0 = max(0, g0 - 1)
            r0 = s0 + 1 - g0
            nsl = g0 + rows - 1 - s0
            nc.scalar.activation(
                out=_ap(cx_sb[:], 64 * 2508 + s0 * 66 + 1, [[38 * 66, 64], [66, nsl], [1, 64]]),
                in_=_ap(pcs[:], r0 * 64, [[512, 64], [64, nsl], [1, 64]]),
                func=AF.Identity, bias=bcomp_sb[:], scale=1.0)

        # ---- 2. offset+mask convs ----
        # expS2/offS2 store interleaved row pairs: slot 2r = row r, slot 2r+1 = row r+16
        # so the [*,128] PE transposes read contiguous pairs. expS rows 16-19 duplicated.
        expS = work.tile([25, 40, 64], F32)
        offS = work.tile([8, 32, 64], F32)

        def eslot(row):  # primary slot for mask row
            return 2 * row if row < 20 else 2 * (row - 16) + 1

        def oslot(row):  # slot for offset row (no duplication)
            return 2 * row if row < 16 else 2 * (row - 16) + 1

        for grp in range(6):
            g0 = grp * 6
            nn = 6 * 64
            pcs = psum.tile([57, 384], F32)
            for dx in range(3):
                rhs = _ap(cx_sb[:], g0 * 66 + dx, [[38 * 66, 128], [66, 6], [1, 64]])
                nc.tensor.matmul(pcs[:, :nn], _ap(wk_sb[:], dx * 57, [[9 * 57, 128], [1, 57]]),
                                 rhs, start=(dx == 0), stop=False)
            for dx in range(3):
                rhs = _ap(cx_sb[:], (g0 + 2) * 66 + dx, [[38 * 66, 64], [66, 6], [1, 64]])
                nc.tensor.matmul(pcs[:, :nn], _ap(wk_sb[:], (6 + dx) * 57, [[9 * 57, 64], [1, 57]]),
                                 rhs, start=False, stop=(dx == 2))
            # evac mask rows to interleaved slots, one ACT per maximal stride-2 run
            pairs = []
            for row in range(g0, g0 + 6):
                if row < 20:
                    pairs.append((row, 2 * row))
                if row >= 16:
                    pairs.append((row, 2 * (row - 16) + 1))
            pairs.sort(key=lambda rs: rs[1])
            runs = []
            for row, s in pairs:
                if runs and runs[-1][0] + 2 * runs[-1][2] == s \
                        and runs[-1][1] + runs[-1][2] == row:
                    runs[-1][2] += 1
                else:
                    runs.append([s, row, 1])
            for s0, r0, n_ in runs:
                nc.scalar.activation(
                    out=_ap(expS[:], s0 * 64, [[2560, 25], [128, n_], [1, 64]]),
                    in_=_ap(pcs[:], 32 * 384 + (r0 - g0) * 64,
                            [[384, 25], [64, n_], [1, 64]]),
                    func=AF.Exp, bias=bco_sb[32:57], scale=1.0)
            lo, hi = max(g0, 2), min(g0 + 6, 34)
            if lo < hi:
                nc.vector.tensor_scalar(
                    out=_ap(offS[:], oslot(lo - 2) * 64, [[2048, 8], [128, hi - lo], [1, 64]]),
                    in0=_ap(pcs[:], (lo - g0) * 64, [[384, 8], [64, hi - lo], [1, 64]]),
                    scalar1=bco_sb[0:8], scalar2=None, op0=OP.add)

        # ---- 3. PE transposes to 128-partition layout (p = 64*jh + w) ----
        # offsets first: pairs (m, m+16) -> deltT2 [128, 16, 8]
        deltT2 = work.tile([128, 128], BF16)
        pt2 = psc.tile([128, 512], F32, name='pt2', tag='pcs')
        for m in range(16):
            nc.tensor.transpose(pt2[:, m * 8:m * 8 + 8],
                                _ap(offS[:], 2 * m * 64, [[2048, 8], [1, 128]]),
                                id_sb[0:8, 0:8])
        nc.scalar.activation(out=deltT2[:], in_=pt2[:, 0:128], func=AF.Copy, scale=1.0)
        # exp: slot pair (2r, 2r+1) -> expT2 [128, 20 hl, 25 k]; hl = mask row - 16jh
        expT2 = work.tile([128, 20, 25], F32)
        pt = psc.tile([128, 512], F32, name='pt', tag='pcs')
        for r in range(20):
            nc.tensor.transpose(pt[:, r * 25:r * 25 + 25],
                                _ap(expS[:], 2 * r * 64, [[2560, 25], [1, 128]]),
                                id_sb[0:25, 0:25])
        nc.scalar.activation(out=expT2[:], in_=pt[:, 0:500], func=AF.Copy, scale=1.0)

        # ---- 4. W9 bilinear-indicator weights [128, 16m, 4u] ----
        def dview(chbase):
            return _ap(deltT2[:], chbase, [[128, 128], [8, 16], [1, 4]])

        def wt(nm):
            return work.tile([128, 64], BF16, name=nm)

        t1, t2 = wt('t1'), wt('t2')
        gxc, x0r, wxt, omwx, x1r = wt('gxc'), wt('x0r'), wt('wxt'), wt('omwx'), wt('x1r')
        gyc, y0r, wyt, omwy, y1r = wt('gyc'), wt('y0r'), wt('wyt'), wt('omwy'), wt('y1r')
        ia, ib = wt('ia'), wt('ib')
        cwx = work.tile([128, 3, 64], BF16)
        rwy = work.tile([128, 3, 64], BF16)
        W9b = work.tile([128, 9 * 64], BF16)

        hrow_bc = _ap(hrow_sb[:], 0, [[16, 128], [1, 16], [0, 4]])
        y63_bc = _ap(y63_sb[:], 0, [[16, 128], [1, 16], [0, 4]])

        def r4(ap):
            return _ap(ap, 0, [[64, 128], [4, 16], [1, 4]])

        nc.vector.tensor_scalar(out=t1[:], in0=dview(0), scalar1=wvec_sb[:], scalar2=None, op0=OP.add)
        nc.vector.tensor_scalar(out=t2[:], in0=t1[:], scalar1=0.0, scalar2=63.0, op0=OP.max, op1=OP.min)
        nc.vector.tensor_scalar(out=gxc[:], in0=t2[:], scalar1=wvec_sb[:], scalar2=None, op0=OP.subtract)
        nc.vector.tensor_scalar(out=x0r[:], in0=gxc[:], scalar1=0.0, scalar2=-1.0, op0=OP.is_lt, op1=OP.mult)
        nc.vector.tensor_tensor(out=wxt[:], in0=gxc[:], in1=x0r[:], op=OP.subtract)
        nc.vector.tensor_scalar(out=omwx[:], in0=wxt[:], scalar1=-1.0, scalar2=1.0, op0=OP.mult, op1=OP.add)
        nc.vector.tensor_scalar(out=x1r[:], in0=x0r[:], scalar1=1.0, scalar2=w63_sb[:], op0=OP.add, op1=OP.min)

        nc.vector.tensor_tensor(out=r4(t1[:]), in0=dview(4), in1=hrow_bc, op=OP.add)
        nc.vector.tensor_scalar(out=t2[:], in0=t1[:], scalar1=0.0, scalar2=63.0, op0=OP.max, op1=OP.min)
        nc.vector.tensor_tensor(out=r4(gyc[:]), in0=r4(t2[:]), in1=hrow_bc, op=OP.subtract)
        nc.vector.tensor_scalar(out=y0r[:], in0=gyc[:], scalar1=0.0, scalar2=-1.0, op0=OP.is_lt, op1=OP.mult)
        nc.vector.tensor_tensor(out=wyt[:], in0=gyc[:], in1=y0r[:], op=OP.subtract)
        nc.vector.tensor_scalar(out=omwy[:], in0=wyt[:], scalar1=-1.0, scalar2=1.0, op0=OP.mult, op1=OP.add)
        nc.vector.tensor_scalar(out=t1[:], in0=y0r[:], scalar1=1.0, scalar2=None, op0=OP.add)
        nc.vector.tensor_tensor(out=r4(y1r[:]), in0=r4(t1[:]), in1=y63_bc, op=OP.min)

        # x0r/y0r in {-1,0}, x1r/y1r in {0,1} always, so the e=-1 weight has only the
        # "0-side" term and e=+1 only the "1-side" term.
        nc.vector.tensor_scalar(out=ia[:], in0=x0r[:], scalar1=-1.0, scalar2=None, op0=OP.is_equal)
        nc.vector.tensor_tensor(out=cwx[:, 0, :], in0=ia[:], in1=omwx[:], op=OP.mult)
        nc.vector.tensor_scalar(out=ia[:], in0=x1r[:], scalar1=1.0, scalar2=None, op0=OP.is_equal)
        nc.vector.tensor_tensor(out=cwx[:, 2, :], in0=ia[:], in1=wxt[:], op=OP.mult)
        nc.vector.tensor_scalar(out=ia[:], in0=x0r[:], scalar1=0.0, scalar2=None, op0=OP.is_equal)
        nc.vector.tensor_scalar(out=ib[:], in0=x1r[:], scalar1=0.0, scalar2=None, op0=OP.is_equal)
        nc.vector.tensor_tensor(out=ia[:], in0=ia[:], in1=omwx[:], op=OP.mult)
        nc.vector.tensor_tensor(out=ib[:], in0=ib[:], in1=wxt[:], op=OP.mult)
        nc.vector.tensor_tensor(out=cwx[:, 1, :], in0=ia[:], in1=ib[:], op=OP.add)
        nc.vector.tensor_scalar(out=ia[:], in0=y0r[:], scalar1=-1.0, scalar2=None, op0=OP.is_equal)
        nc.vector.tensor_tensor(out=rwy[:, 0, :], in0=ia[:], in1=omwy[:], op=OP.mult)
        nc.vector.tensor_scalar(out=ia[:], in0=y1r[:], scalar1=1.0, scalar2=None, op0=OP.is_equal)
        nc.vector.tensor_tensor(out=rwy[:, 2, :], in0=ia[:], in1=wyt[:], op=OP.mult)
        nc.vector.tensor_scalar(out=ia[:], in0=y0r[:], scalar1=0.0, scalar2=None, op0=OP.is_equal)
        nc.vector.tensor_scalar(out=ib[:], in0=y1r[:], scalar1=0.0, scalar2=None, op0=OP.is_equal)
        nc.vector.tensor_tensor(out=ia[:], in0=ia[:], in1=omwy[:], op=OP.mult)
        nc.vector.tensor_tensor(out=ib[:], in0=ib[:], in1=wyt[:], op=OP.mult)
        nc.vector.tensor_tensor(out=rwy[:, 1, :], in0=ia[:], in1=ib[:], op=OP.add)
        for iy in range(3):
            for ix in range(3):
                nc.vector.tensor_tensor(
                    out=_ap(W9b[:], (iy * 3 + ix) * 64, [[9 * 64, 128], [1, 64]]),
                    in0=rwy[:, iy, :], in1=cwx[:, ix, :], op=OP.mult)

        # ---- 5. softmax (u-expanded) + shifted variants ----
        sumT = work.tile([128, 20], F32)
        nc.vector.tensor_reduce(out=sumT[:], in_=expT2[:], axis=mybir.AxisListType.X, op=OP.add)
        recT = work.tile([128, 20], F32)
        nc.vector.reciprocal(out=recT[:], in_=sumT[:])
        msm4 = work.tile([128, 500], BF16)   # [128, 20 hl, 25 k]; u broadcast via 0-stride
        nc.vector.tensor_tensor(
            out=_ap(msm4[:], 0, [[500, 128], [25, 20], [1, 25]]),
            in0=_ap(expT2[:], 0, [[500, 128], [25, 20], [1, 25]]),
            in1=_ap(recT[:], 0, [[20, 128], [1, 20], [0, 25]]), op=OP.mult)
        nc.sync.dma_start(out=_ap(msm4_p1[:], 0, [[500, 127], [1, 500]]),
                          in_=_ap(msm4[:], 500, [[500, 127], [1, 500]]))
        nc.scalar.dma_start(out=_ap(msm4_m1[:], 500, [[500, 127], [1, 500]]),
                            in_=_ap(msm4[:], 0, [[500, 127], [1, 500]]))

        # ---- 5-9 software-pipelined by m-groups ----
        kernc = work.tile([128, 1600], BF16)   # [128, 16 m, 25 k, 4 u] (k is kx-major)
        tmpA = work.tile([128, 1600], BF16)
        tmpB = work.tile([128, 1600], BF16)
        data_all = work.tile([128, 16, 100], BF16)
        msm_by_ex = {-1: msm4_m1, 0: msm4, 1: msm4_p1}
        NG = 8
        GM = 16 // NG
        # center/ex=0 terms first so the msm4 shift DMAs can complete in their shadow
        E_ORDER = [(0, 1), (1, 1), (2, 1), (0, 0), (1, 0), (2, 0), (0, 2), (1, 2), (2, 2)]

        def emit_asm(G):
            # interleave M0 M1 M2 A1 M3 A2 ... (two tmp tiles) to keep RAW deps
            # >= 2 ops apart and hide the DVE pipeline latency
            def kv():
                return _ap(kernc[:], GM * G * 100, [[1600, 128], [100, GM], [4, 25], [1, 4]])

            def tv(t_):
                return _ap(t_[:], GM * G * 100, [[1600, 128], [100, GM], [4, 25], [1, 4]])

            def term(n_):
                iy, ix = E_ORDER[n_]
                ey, ex = iy - 1, ix - 1
                mv = _ap(msm_by_ex[ex][:], (2 + ey + GM * G) * 25,
                         [[500, 128], [25, GM], [1, 25], [0, 4]])
                wv = _ap(W9b[:], (iy * 3 + ix) * 64 + GM * G * 4,
                         [[9 * 64, 128], [4, GM], [0, 25], [1, 4]])
                return mv, wv

            mv, wv = term(0)
            nc.vector.tensor_tensor(out=kv(), in0=wv, in1=mv, op=OP.mult)  # M0
            tms = [tmpA, tmpB]
            for j in (1, 2):                                               # M1 M2
                mv, wv = term(j)
                nc.vector.tensor_tensor(out=tv(tms[j - 1]), in0=wv, in1=mv, op=OP.mult)
            for j in range(1, 9):                                          # A_j (+ M_{j+2})
                nc.vector.tensor_tensor(out=kv(), in0=kv(), in1=tv(tms[(j - 1) % 2]), op=OP.add)
                if j + 2 <= 8:
                    mv, wv = term(j + 2)
                    nc.vector.tensor_tensor(out=tv(tms[(j + 1) % 2]), in0=wv, in1=mv, op=OP.mult)

        def emit_dmas(G):
            # kbf[d][p] = kernc[p+d], full 128-partition range: cross-block rows are
            # skipped via -1 idx entries, end partitions stay zero from the memset
            for d in (1, -1, 2, -2):
                cnt = 128 - abs(d)
                eng = nc.sync if d > 0 else nc.scalar
                oo = (-d if d < 0 else 0) * 1600 + GM * G * 100
                io = (d if d > 0 else 0) * 1600 + GM * G * 100
                eng.dma_start(
                    out=_ap(kbf[d][:], oo, [[1600, cnt], [1, GM * 100]]),
                    in_=_ap(kernc[:], io, [[1600, cnt], [1, GM * 100]]))

        def emit_prep(G):
            # data_all[p, m, b*20+ky*4+u] = kernc[p+b-2, m, kx=4-b, ky, u]
            for b in (2, 1, 3, 0, 4):   # b=2 reads kernc directly (no DMA dependency)
                src = kernc if b == 2 else kbf[b - 2]
                nc.vector.tensor_copy(
                    out=_ap(data_all[:], GM * G * 100 + b * 20, [[1600, 128], [100, GM], [1, 20]]),
                    in_=_ap(src[:], GM * G * 100 + (4 - b) * 20, [[1600, 128], [100, GM], [1, 20]]))

        def emit_pairs(G):
            for m in range(GM * G, GM * G + GM):
                banded1 = band.tile([128, 1536], BF16, name=f'band1_{m}', tag='band1')
                banded2 = band.tile([128, 1024], BF16, name=f'band2_{m}', tag='band2')
                nc.gpsimd.local_scatter(out_ap=banded1[:], data_ap=data_all[:, m, :],
                                        idxs_ap=idx1_sb[:], channels=128, num_elems=1536, num_idxs=100)
                nc.gpsimd.local_scatter(out_ap=banded2[:], data_ap=data_all[:, m, :],
                                        idxs_ap=idx2_sb[:], channels=128, num_elems=1024, num_idxs=100)
                for ch in range(2):
                    pcs = psc.tile([128, 512], F32, name=f'pcs_{m}_{ch}', tag='pcs')
                    for ki in range(5):
                        lhsT = _ap(xT2_sb[:], (m + ki) * 256 + ch * 128, [[20 * 256, 128], [1, 128]])
                        rhs = banded1[:, ki * 512:ki * 512 + 512] if ki < 3 \
                            else banded2[:, (ki - 3) * 512:(ki - 3) * 512 + 512]
                        nc.tensor.matmul(pcs[:], lhsT, rhs, start=(ki == 0), stop=(ki == 4))
                    rb = rowp.tile([128, 512], BF16, name=f'rb_{m}_{ch}', tag='rb')
                    nc.scalar.activation(out=rb[:], in_=pcs[:], func=AF.Copy, scale=1.0)
                    nc.sync.dma_start(
                        out=_ap(outp[:], ch * 128 * 8192 + 4 * m * 128,
                                [[8192, 128], [128, 4], [1, 128]]),
                        in_=rb[:])

        emit_asm(0)
        emit_dmas(0)
        emit_prep(0)
        for G in range(NG):
            if G + 1 < NG:
                emit_asm(G + 1)
                emit_dmas(G + 1)
                emit_prep(G + 1)
            emit_pairs(G)
    nc.finalize()
    return nc


_PROGRAM = None
_SCAT = build_scatter_tables()


def _get_program():
    global _PROGRAM
    if _PROGRAM is None:
        _PROGRAM = build_program()
    return _PROGRAM


def _prep_core_inputs(inputs, n, s):
    bf = np.float16
    x = np.asarray(inputs['x'][n], np.float32)
    h0 = 32 * s
    xw = np.zeros((C, 38, W), np.float32)
    for i, g in enumerate(range(h0 - 3, h0 + 35)):
        if 0 <= g < H:
            xw[:, i] = x[:, g]
    xwin = np.ascontiguousarray(xw.reshape(2, 128, 38 * 64)).astype(bf)
    xT2 = np.zeros((128, 20, C), np.float32)
    for jh in range(2):
        base = h0 + 16 * jh - 2
        for i in range(20):
            g = base + i
            if 0 <= g < H:
                xT2[64 * jh:64 * jh + 64, i] = x[:, g].T
    xT2 = np.ascontiguousarray(xT2.reshape(128, 20 * 256)).astype(bf)
    w_comp = np.asarray(inputs['w_comp'], np.float32)[:, :, 0, 0]
    wc = np.zeros((2, 128, 64), np.float32)
    for cg in range(2):
        wc[cg] = w_comp[:, cg * 128:(cg + 1) * 128].T
    wc = np.ascontiguousarray(wc.transpose(1, 0, 2).reshape(128, 2 * 64)).astype(bf)
    w_ker = np.asarray(inputs['w_ker'], np.float32)
    w_off = np.asarray(inputs['w_off'], np.float32)
    # mask channels permuted kx-major: new k = kx*5+ky holds w_ker[ky*5+kx]
    kperm = np.array([(k % 5) * 5 + k // 5 for k in range(25)])
    wk = np.zeros((9, 64, 57), np.float32)
    for t in range(9):
        wk[t, :, 0:8] = w_off[:, :, t // 3, t % 3].T
        wk[t, :, 32:57] = w_ker[kperm, :, t // 3, t % 3].T
    # [128, 9*57]: top half = tap t (dy=t//3), bottom half = tap t+3 (dy+1) for t<3
    wk2 = np.zeros((128, 9, 57), np.float32)
    wk2[0:64] = wk.transpose(1, 0, 2)
    wk2[64:128, 0:3] = wk[3:6].transpose(1, 0, 2)
    wk = np.ascontiguousarray(wk2.reshape(128, 9 * 57)).astype(bf)
    bcov = np.zeros((57, 1), np.float32)
    bcov[0:8, 0] = np.asarray(inputs['b_off'], np.float32)
    bcov[32:57, 0] = np.asarray(inputs['b_ker'], np.float32)[kperm]
    idx1, idx2 = _SCAT
    pp = np.arange(128, dtype=np.float32)
    hr = (h0 + 16.0 * (pp // 64))[:, None] + np.arange(16, dtype=np.float32)[None, :]
    return {
        'xwin': xwin, 'xT2': xT2, 'wc': wc, 'wk': wk, 'bco': bcov,
        'bcomp': np.asarray(inputs['b_comp'], np.float32).reshape(64, 1),
        'wvec': (pp % 64).reshape(128, 1),
        'w63': (63.0 - pp % 64).reshape(128, 1),
        'hrow': np.ascontiguousarray(hr),
        'y63': np.ascontiguousarray(63.0 - hr),
        'ident': np.eye(128, dtype=np.float32),
        'idx1': idx1, 'idx2': idx2,
    }


def kernel(**inputs):
    nc = _get_program()
    core_ids = list(range(8))
    in_maps = [_prep_core_inputs(inputs, cid // 2, cid % 2) for cid in core_ids]
    res = run_bass_kernel_spmd(nc, in_maps, core_ids)
    out = np.zeros((N, C, 128, 128), np.float32)
    for cid in core_ids:
        n, s = cid // 2, cid % 2
        op = np.asarray(res.results[cid]['outp']).reshape(256, 64, 128)
        out[n, :, s::2] = op
    return out


if __name__ == '__main__':
    d = np.load('/root/problem/ref_io.npz')
    inp = {k: d[k] for k in ('x', 'w_comp', 'b_comp', 'w_ker', 'b_ker', 'w_off', 'b_off')}
    out = kernel(**inp)
    ref = d['out']
    err = np.abs(out - ref).max()
    print('max abs err:', err, 'rel:', err / np.abs(ref).max())
